# revision 1
# baseline (speedup 1.0000x reference)
"""Trainium2 Bass kernel for CdfgReader GNN message passing.

Strategy:
  - The GNN node features depend only on which CDFG a batch item references.
    With 64 batch items drawn from 32 CDFGs, compute the GNN once per UNIQUE
    graph (<=32) and distribute 4 graph slots per core across 8 cores.
  - Per graph slot: X0 = relu(xs @ W_in + b), 4 GCN layers
    (A @ (X @ W) + b with relu/tanh), residual, then per-batch masked mean
    via a small mask matmul. Each core emits the [64, 256] rows for the
    batch items whose graph it owns; the host gathers rows from owners.
  - Precision: X and W stay fp32 (fp32 matmuls for the small X@W work —
    rounding W to f32r alone costs 2.4e-2 end-to-end error). The dominant
    A-multiplies run in float32r (full PE rate): A is 0/1 (exact in f32r)
    and XW is split into hi+lo f32r parts on layers 0-2 so the product is
    fp32-accurate; layer 3 uses hi only. Measured end-to-end ~5e-5.
  - A^T is pre-transposed on the host (the PE contracts over the partition
    dim, and fp32 has no DMA-transpose path on TRN2).
"""

import os

import numpy as np

NG = 4          # graph slots per core
NCORES = 8
N = 1024        # max nodes
F = 128         # input feature dim
H = 256         # hidden dim
L = 4           # GCN layers
B = 64          # batch (coverpoints)

SPLIT_LAYERS = (0, 1, 2)   # A-mult layers using hi+lo split

_CACHE = {}


def _build_nc():
    import concourse.bass as bass  # noqa: F401
    import concourse.mybir as mybir
    import concourse.tile as tile
    from concourse import bacc
    from concourse.bass import ts

    f32 = mybir.dt.float32
    f32r = mybir.dt.float32r
    Relu = mybir.ActivationFunctionType.Relu
    Tanh = mybir.ActivationFunctionType.Tanh
    sub = mybir.AluOpType.subtract

    nc = bacc.Bacc("TRN2", target_bir_lowering=False, debug=False,
                   num_devices=NCORES)

    a_t = nc.dram_tensor("a_t", [NG, N, N], f32r, kind="ExternalInput")
    xs_t_hi = nc.dram_tensor("xs_t_hi", [F, NG, N], f32r, kind="ExternalInput")
    xs_t_lo = nc.dram_tensor("xs_t_lo", [F, NG, N], f32r, kind="ExternalInput")
    m_t = nc.dram_tensor("m_t", [128, NG * 8, B], f32r, kind="ExternalInput")
    mask_full = nc.dram_tensor("mask_full", [B, N], f32, kind="ExternalInput")
    w_in_hi = nc.dram_tensor("w_in_hi", [F, H], f32r, kind="ExternalInput")
    w_in_lo = nc.dram_tensor("w_in_lo", [F, H], f32r, kind="ExternalInput")
    w_gcn_hi = nc.dram_tensor("w_gcn_hi", [128, L * 2, H], f32r,
                              kind="ExternalInput")
    w_gcn_lo = nc.dram_tensor("w_gcn_lo", [128, L * 2, H], f32r,
                              kind="ExternalInput")
    b_in_pp = nc.dram_tensor("b_in_pp", [128, 2], f32, kind="ExternalInput")
    b_gcn_pp = nc.dram_tensor("b_gcn_pp", [128, L * 2], f32, kind="ExternalInput")
    b_in_row = nc.dram_tensor("b_in_row", [1, H], f32r, kind="ExternalInput")
    b_g3_row = nc.dram_tensor("b_g3_row", [1, H], f32r, kind="ExternalInput")
    ones_row = nc.dram_tensor("ones_row", [1, 128], f32r, kind="ExternalInput")
    out = nc.dram_tensor("out", [B, H], f32, kind="ExternalOutput")

    with tile.TileContext(nc) as tc:
        with (
            tc.tile_pool(name="const", bufs=1) as constp,
            tc.tile_pool(name="apool", bufs=2) as apool,
            tc.tile_pool(name="xpool", bufs=2) as xpool,
            tc.tile_pool(name="xpool1", bufs=1) as xpool1,
            tc.tile_pool(name="psx", bufs=4, space="PSUM") as psx,
            tc.tile_pool(name="psw", bufs=3, space="PSUM") as psw,
            tc.tile_pool(name="psm", bufs=1, space="PSUM") as psm,
        ):
            # --- constants, loaded once ---
            wi_hi_sb = constp.tile([128, H], f32r)
            nc.sync.dma_start(wi_hi_sb[:], w_in_hi[:, :])
            wi_lo_sb = constp.tile([128, H], f32r)
            nc.sync.dma_start(wi_lo_sb[:], w_in_lo[:, :])
            w_hi_sb = constp.tile([128, L * 2, H], f32r)
            nc.sync.dma_start(w_hi_sb[:], w_gcn_hi[:, :, :])
            w_lo_sb = constp.tile([128, L * 2, H], f32r)
            nc.sync.dma_start(w_lo_sb[:], w_gcn_lo[:, :, :])
            b_in_pp_sb = constp.tile([128, 2], f32)
            nc.sync.dma_start(b_in_pp_sb[:], b_in_pp[:, :])
            b_gcn_pp_sb = constp.tile([128, L * 2], f32)
            nc.sync.dma_start(b_gcn_pp_sb[:], b_gcn_pp[:, :])
            b_in_row_sb = constp.tile([1, H], f32r)
            nc.sync.dma_start(b_in_row_sb[:], b_in_row[:, :])
            b_g3_row_sb = constp.tile([1, H], f32r)
            nc.sync.dma_start(b_g3_row_sb[:], b_g3_row[:, :])
            ones_sb = constp.tile([1, 128], f32r)
            nc.sync.dma_start(ones_sb[:], ones_row[:, :])
            m_t_sb = constp.tile([128, NG * 8, B], f32r)
            nc.sync.dma_start(m_t_sb[:], m_t[:, :, :])

            out_acc = constp.tile([B, H], f32)

            for g in range(NG):
                # A^T for this graph: 8 tiles [128(m), 1024(i)] in one tensor
                a_sb = apool.tile([128, 8, N], f32r, tag="a")
                nc.sync.dma_start(
                    a_sb[:], a_t[g].rearrange("(mo p) i -> p mo i", p=128))
                xs_g_hi = xpool.tile([128, N], f32r, tag="xs_g_hi")
                nc.sync.dma_start(xs_g_hi[:], xs_t_hi[:, g, :])
                xs_g_lo = xpool.tile([128, N], f32r, tag="xs_g_lo")
                nc.sync.dma_start(xs_g_lo[:], xs_t_lo[:, g, :])

                # X0^T hi/lo f32r companions (h-major) feed the split X@W
                # matmuls; the fp32 value only lives in a transient chunk.
                x0t_hi = xpool.tile([128, 2, N], f32r, tag="xh", name="x0t_hi")
                x0t_lo = xpool.tile([128, 2, N], f32r, tag="xl", name="x0t_lo")
                for t in range(2):
                    for c in range(2):
                        ps = psx.tile([128, 512], mybir.dt.float32, tag="psx")
                        for k, (lhsT, rhs) in enumerate(
                                ((wi_hi_sb[:, ts(t, 128)], xs_g_hi[:, ts(c, 512)]),
                                 (wi_lo_sb[:, ts(t, 128)], xs_g_hi[:, ts(c, 512)]),
                                 (wi_hi_sb[:, ts(t, 128)], xs_g_lo[:, ts(c, 512)]))):
                            nc.tensor.matmul(ps[:], lhsT, rhs,
                                             start=(k == 0), stop=(k == 2))
                        xtmp = xpool.tile([128, 512], f32, tag="xtmp",
                                          name="x0tmp")
                        nc.scalar.activation(xtmp[:], ps[:],
                                             Relu, bias=b_in_pp_sb[:, t:t + 1])
                        nc.vector.tensor_copy(x0t_hi[:, t, ts(c, 512)],
                                              xtmp[:])
                        nc.vector.tensor_tensor(x0t_lo[:, t, ts(c, 512)],
                                                xtmp[:],
                                                x0t_hi[:, t, ts(c, 512)], sub)

                # X0 node-major fp32 (for the residual): [128, 8(i), 256(h)]
                x0n = xpool.tile([128, 8, H], f32, tag="x0n")
                for i in range(8):
                    ps = psw.tile([128, H], mybir.dt.float32, tag="psw")
                    for k, (lhsT, rhs) in enumerate(
                            ((xs_g_hi[:, ts(i, 128)], wi_hi_sb[:]),
                             (xs_g_hi[:, ts(i, 128)], wi_lo_sb[:]),
                             (xs_g_lo[:, ts(i, 128)], wi_hi_sb[:]))):
                        nc.tensor.matmul(ps[:], lhsT, rhs,
                                         start=(k == 0), stop=False)
                    nc.tensor.matmul(ps[:], ones_sb[:], b_in_row_sb[:],
                                     start=False, stop=True)
                    nc.scalar.activation(x0n[:, i, :], ps[:], Relu)

                x_hi, x_lo = x0t_hi, x0t_lo
                xf = None
                for layer in range(L):
                    do_split = layer in SPLIT_LAYERS
                    # XW = X @ W_gcn[layer] via 3-way f32r split
                    # (X_hi@W_hi + X_lo@W_hi + X_hi@W_lo), then round/split
                    xw_hi = xpool.tile([128, 8, H], f32r, tag="xw_hi",
                                       name="xw_hi")
                    xw_lo = None
                    if do_split:
                        xw_lo = xpool1.tile([128, 8, H], f32r, tag="xw_lo",
                                            name="xw_lo")
                    for m in range(8):
                        ps = psw.tile([128, H], mybir.dt.float32, tag="psw")
                        k = 0
                        for t in range(2):
                            wh = w_hi_sb[:, layer * 2 + t, :]
                            wl = w_lo_sb[:, layer * 2 + t, :]
                            for lhsT, rhs in ((x_hi[:, t, ts(m, 128)], wh),
                                              (x_hi[:, t, ts(m, 128)], wl),
                                              (x_lo[:, t, ts(m, 128)], wh)):
                                nc.tensor.matmul(ps[:], lhsT, rhs,
                                                 start=(k == 0), stop=(k == 5))
                                k += 1
                        nc.vector.tensor_copy(xw_hi[:, m, :], ps[:])
                        if do_split:
                            nc.vector.tensor_tensor(
                                xw_lo[:, m, :], ps[:], xw_hi[:, m, :], sub)

                    parts = [xw_hi, xw_lo] if do_split else [xw_hi]
                    if layer < L - 1:
                        # X_next^T[h, i] = sum_m XW[m, h] * A^T[m, i]  (h-major)
                        xn_hi = xpool.tile([128, 2, N], f32r, tag="xh",
                                           name="xn_hi")
                        xn_lo = xpool.tile([128, 2, N], f32r, tag="xl",
                                           name="xn_lo")
                        for t in range(2):
                            pss = [psx.tile([128, 512], mybir.dt.float32,
                                            tag="psx", name=f"ps_{t}_{c}")
                                   for c in range(2)]
                            nmm = 8 * len(parts)
                            k = 0
                            for m in range(8):
                                for part in parts:
                                    for c in range(2):
                                        nc.tensor.matmul(
                                            pss[c][:], part[:, m, ts(t, 128)],
                                            a_sb[:, m, ts(c, 512)],
                                            start=(k == 0), stop=(k == nmm - 1))
                                    k += 1
                            for c in range(2):
                                xtmp = xpool.tile([128, 512], f32, tag="xtmp",
                                                  name="xtmp")
                                nc.scalar.activation(
                                    xtmp[:], pss[c][:], Relu,
                                    bias=b_gcn_pp_sb[:, layer * 2 + t:
                                                     layer * 2 + t + 1])
                                nc.vector.tensor_copy(
                                    xn_hi[:, t, ts(c, 512)], xtmp[:])
                                nc.vector.tensor_tensor(
                                    xn_lo[:, t, ts(c, 512)], xtmp[:],
                                    xn_hi[:, t, ts(c, 512)], sub)
                        x_hi, x_lo = xn_hi, xn_lo
                    else:
                        # Final layer node-major: X4[i, h] = sum_m A^T[m,i]^T XW[m,h]
                        xf = xpool1.tile([128, 8, H], f32r, tag="xf")
                        for i in range(8):
                            ps = psw.tile([128, H], mybir.dt.float32, tag="psw")
                            for m in range(8):
                                for part in parts:
                                    nc.tensor.matmul(
                                        ps[:], a_sb[:, m, ts(i, 128)],
                                        part[:, m, :],
                                        start=(m == 0 and part is parts[0]),
                                        stop=False)
                            nc.tensor.matmul(ps[:], ones_sb[:], b_g3_row_sb[:],
                                             start=False, stop=True)
                            nc.scalar.activation(ps[:], ps[:], Tanh)
                            # residual add; output rounds to f32r for mask mm
                            nc.vector.tensor_add(xf[:, i, :], ps[:],
                                                 x0n[:, i, :])

                # masked sums for the batch rows owned via this graph:
                # psum[b, h] += M^T[n, b]^T @ Xf[n, h]
                pm = psm.tile([B, H], mybir.dt.float32, tag="psm")
                for c in range(8):
                    nc.tensor.matmul(pm[:], m_t_sb[:, g * 8 + c, :],
                                     xf[:, c, :], start=(c == 0), stop=(c == 7))
                if g == 0:
                    nc.vector.tensor_copy(out_acc[:], pm[:])
                else:
                    nc.vector.tensor_add(out_acc[:], out_acc[:], pm[:])

            # --- epilogue: divide by per-batch mask count ---
            mask_sb = constp.tile([B, N], f32)
            nc.sync.dma_start(mask_sb[:], mask_full[:, :])
            cnt = constp.tile([B, 1], f32)
            nc.vector.reduce_sum(cnt[:], mask_sb[:], axis=mybir.AxisListType.X)
            inv = constp.tile([B, 1], f32)
            nc.vector.reciprocal(inv[:], cnt[:])
            out_sb = constp.tile([B, H], f32)
            nc.vector.tensor_scalar_mul(out_sb[:], out_acc[:], inv[:])
            nc.sync.dma_start(out[:, :], out_sb[:])

    nc.compile()
    return nc


def _get_nc():
    if "nc" not in _CACHE:
        _CACHE["nc"] = _build_nc()
    return _CACHE["nc"]


def _prepare_in_maps(cdfg_xs, cdfg_as, graph, coverpoint_mask,
                     W_in, b_in, W_gcn, b_gcn):
    cdfg_xs = np.asarray(cdfg_xs, dtype=np.float32)
    cdfg_as = np.asarray(cdfg_as, dtype=np.float32)
    graph = np.asarray(graph).astype(np.int64)
    maskf = np.asarray(coverpoint_mask).astype(np.float32)
    W_in = np.asarray(W_in, dtype=np.float32)
    b_in = np.asarray(b_in, dtype=np.float32)
    W_gcn = np.asarray(W_gcn, dtype=np.float32)
    b_gcn = np.asarray(b_gcn, dtype=np.float32)

    uniq = np.unique(graph)
    nslots = NG * NCORES
    slots = np.empty(nslots, dtype=np.int64)
    slots[:len(uniq)] = uniq
    slots[len(uniq):] = uniq[0]
    real = np.zeros(nslots, dtype=bool)
    real[:len(uniq)] = True

    def _rnd11(x):
        # round-to-nearest-even at 11 explicit mantissa bits (f32r-exact)
        m, e = np.frexp(np.float32(x))
        m = np.round(m * 4096.0) / 4096.0
        return np.ldexp(m, e).astype(np.float32)

    w_gcn_layout = np.ascontiguousarray(
        W_gcn.reshape(L, 2, 128, H).transpose(2, 0, 1, 3)
        .reshape(128, L * 2, H))
    w_gcn_hi = _rnd11(w_gcn_layout)
    w_gcn_lo = _rnd11(w_gcn_layout - w_gcn_hi)
    w_in_hi = _rnd11(W_in)
    w_in_lo = _rnd11(W_in - w_in_hi)

    common = {
        "w_in_hi": np.ascontiguousarray(w_in_hi),
        "w_in_lo": np.ascontiguousarray(w_in_lo),
        "w_gcn_hi": w_gcn_hi,
        "w_gcn_lo": w_gcn_lo,
        "b_in_pp": np.ascontiguousarray(b_in.reshape(2, 128).T),
        "b_gcn_pp": np.ascontiguousarray(
            b_gcn.reshape(L, 2, 128).transpose(2, 0, 1).reshape(128, L * 2)),
        "b_in_row": np.ascontiguousarray(b_in.reshape(1, H)),
        "b_g3_row": np.ascontiguousarray(b_gcn[L - 1].reshape(1, H)),
        "ones_row": np.ones((1, 128), dtype=np.float32),
        "mask_full": np.ascontiguousarray(maskf),
    }

    in_maps = []
    for k in range(NCORES):
        sl = slots[k * NG:(k + 1) * NG]
        a_t = np.empty((NG, N, N), dtype=np.float32)
        for g in range(NG):
            a_t[g] = cdfg_as[sl[g]].T
        xs_t = np.ascontiguousarray(cdfg_xs[sl].transpose(2, 0, 1))
        xs_t_hi = _rnd11(xs_t)
        xs_t_lo = _rnd11(xs_t - xs_t_hi)
        m_t = np.zeros((128, NG * 8, B), dtype=np.float32)
        for g in range(NG):
            if real[k * NG + g]:
                rows = np.nonzero(graph == sl[g])[0]
                for b in rows:
                    m_t[:, g * 8:(g + 1) * 8, b] = maskf[b].reshape(8, 128).T
        in_maps.append({"a_t": a_t, "xs_t_hi": xs_t_hi, "xs_t_lo": xs_t_lo,
                        "m_t": m_t, **common})
    return in_maps, slots, real


def _assemble_out(results, graph, slots, real):
    graph = np.asarray(graph).astype(np.int64)
    out = np.zeros((B, H), dtype=np.float32)
    for k in range(NCORES):
        for g in range(NG):
            if real[k * NG + g]:
                rows = graph == slots[k * NG + g]
                out[rows] = results[k]["out"][rows]
    return out


def kernel(cdfg_xs, cdfg_as, graph, coverpoint_mask, W_in, b_in, W_gcn, b_gcn):
    from concourse.bass_utils import run_bass_kernel_spmd

    nc = _get_nc()
    in_maps, slots, real = _prepare_in_maps(
        cdfg_xs, cdfg_as, graph, coverpoint_mask, W_in, b_in, W_gcn, b_gcn)
    res = run_bass_kernel_spmd(nc, in_maps, core_ids=list(range(NCORES)))
    return _assemble_out(res.results, graph, slots, real)



# revision 7
# speedup vs baseline: 1.6887x; 1.6887x over previous
"""Trainium2 Bass kernel for CdfgReader GNN message passing.

Strategy:
  - 64 batch items draw from <=32 unique CDFGs: compute the GNN once per
    unique graph; distribute ceil(u/8) graph slots per core across 8 cores
    (SPMD, one compiled program specialized to the input's structure).
  - Error budget (tolerance 2e-2): the end-to-end error is dominated by the
    f32r rounding of the *weights* (a systematic perturbation); activation
    rounding averages out through the A-multiply and the masked mean.
    So W_in/W_gcn ship as f32r hi+lo pairs (every X@W does 2 matmuls per
    contraction tile), while activations stay single f32r and every A-mult
    runs once.  Measured end-to-end ~1.2e-3.
  - Per slot: X0^T = relu(W^T xs^T) h-major; 3x { XW node-major;
    X^T = relu(XW^T A^T) h-major }; final layer computed node-major only
    for the first K_g 128-node tiles, where the host permutes each graph's
    nodes so the union of its coverpoint masks comes first.  The residual
    (X0) is re-materialized node-major with PE transposes, and the masked
    sums use a small mask matmul.  All matmuls stream at 1 row/cycle
    (f32r everywhere; the NEFF compiler forbids mixing 32-bit with 16/8-bit
    matmul operands, so A/mask/identity stay f32r too).
"""

import numpy as np

NCORES = 8
N = 1024        # max nodes
F = 128         # input feature dim
H = 256         # hidden dim
L = 4           # GCN layers
B = 64          # batch (coverpoints)

_CACHE = {}


def _rnd11(x):
    # round-to-nearest-even at 11 explicit mantissa bits (f32r-exact)
    m, e = np.frexp(np.float32(x))
    m = np.round(m * 4096.0) / 4096.0
    return np.ldexp(m, e).astype(np.float32)


def _build_nc(NG, Ks):
    import concourse.bass as bass  # noqa: F401
    import concourse.mybir as mybir
    import concourse.tile as tile
    from concourse import bacc
    from concourse.bass import ts

    f32 = mybir.dt.float32
    f32r = mybir.dt.float32r
    Relu = mybir.ActivationFunctionType.Relu
    Tanh = mybir.ActivationFunctionType.Tanh
    add = mybir.AluOpType.add

    T = sum(Ks)
    offs = np.concatenate([[0], np.cumsum(Ks)]).astype(int)
    Kmax = max(Ks)

    nc = bacc.Bacc("TRN2", target_bir_lowering=False, debug=False,
                   num_devices=NCORES)

    a_t = nc.dram_tensor("a_t", [NG, N, N], f32r, kind="ExternalInput")
    xs_t = nc.dram_tensor("xs_t", [F, NG, N], f32r, kind="ExternalInput")
    m_t = nc.dram_tensor("m_t", [128, T, B], f32r, kind="ExternalInput")
    w_in_hi = nc.dram_tensor("w_in_hi", [F, H], f32r, kind="ExternalInput")
    w_in_lo = nc.dram_tensor("w_in_lo", [F, H], f32r, kind="ExternalInput")
    w_gcn_hi = nc.dram_tensor("w_gcn_hi", [128, L * 2, H], f32r,
                              kind="ExternalInput")
    w_gcn_lo = nc.dram_tensor("w_gcn_lo", [128, L * 2, H], f32r,
                              kind="ExternalInput")
    b_in_pp = nc.dram_tensor("b_in_pp", [128, 2], f32, kind="ExternalInput")
    b_gcn_pp = nc.dram_tensor("b_gcn_pp", [128, L * 2], f32,
                              kind="ExternalInput")
    b_g3_row = nc.dram_tensor("b_g3_row", [1, H], f32r, kind="ExternalInput")
    ones_row = nc.dram_tensor("ones_row", [1, 128], f32r, kind="ExternalInput")
    ident = nc.dram_tensor("ident", [128, 128], f32r, kind="ExternalInput")
    inv_cnt = nc.dram_tensor("inv_cnt", [B, 1], f32, kind="ExternalInput")
    out = nc.dram_tensor("out", [B, H], f32, kind="ExternalOutput")

    with tile.TileContext(nc) as tc:
        with (
            tc.tile_pool(name="const", bufs=1) as constp,
            tc.tile_pool(name="apool", bufs=2) as apool,
            tc.tile_pool(name="xspool", bufs=2) as xspool,
            tc.tile_pool(name="x0pool", bufs=2) as x0pool,
            tc.tile_pool(name="xpool", bufs=2) as xpool,
            tc.tile_pool(name="xwpool", bufs=2) as xwpool,
            tc.tile_pool(name="xfpool", bufs=2) as xfpool,
            tc.tile_pool(name="psx", bufs=3, space="PSUM") as psx,
            tc.tile_pool(name="psw", bufs=2, space="PSUM") as psw,
            tc.tile_pool(name="pst", bufs=2, space="PSUM") as pstp,
            tc.tile_pool(name="psm", bufs=1, space="PSUM") as psm,
        ):
            # --- constants, loaded once ---
            wi_hi_sb = constp.tile([128, H], f32r)
            nc.sync.dma_start(wi_hi_sb[:], w_in_hi[:, :])
            wi_lo_sb = constp.tile([128, H], f32r)
            nc.sync.dma_start(wi_lo_sb[:], w_in_lo[:, :])
            w_hi_sb = constp.tile([128, L * 2, H], f32r)
            nc.sync.dma_start(w_hi_sb[:], w_gcn_hi[:, :, :])
            w_lo_sb = constp.tile([128, L * 2, H], f32r)
            nc.sync.dma_start(w_lo_sb[:], w_gcn_lo[:, :, :])
            b_in_pp_sb = constp.tile([128, 2], f32)
            nc.sync.dma_start(b_in_pp_sb[:], b_in_pp[:, :])
            b_gcn_pp_sb = constp.tile([128, L * 2], f32)
            nc.sync.dma_start(b_gcn_pp_sb[:], b_gcn_pp[:, :])
            b_g3_row_sb = constp.tile([1, H], f32r)
            nc.sync.dma_start(b_g3_row_sb[:], b_g3_row[:, :])
            ones_sb = constp.tile([1, 128], f32r)
            nc.sync.dma_start(ones_sb[:], ones_row[:, :])
            ident_sb = constp.tile([128, 128], f32r)
            nc.sync.dma_start(ident_sb[:], ident[:, :])
            m_t_sb = constp.tile([128, T, B], f32r)
            nc.sync.dma_start(m_t_sb[:], m_t[:, :, :])
            inv_sb = constp.tile([B, 1], f32)
            nc.sync.dma_start(inv_sb[:], inv_cnt[:, :])

            out_acc = constp.tile([B, H], f32)

            for g in range(NG):
                K = Ks[g]
                off = int(offs[g])
                # A^T for this graph: 8 tiles [128(m), 1024(i)]
                a_sb = apool.tile([128, 8, N], f32r, tag="a")
                nc.sync.dma_start(
                    a_sb[:], a_t[g].rearrange("(mo p) i -> p mo i", p=128))
                xs_g = xspool.tile([128, N], f32r, tag="xs")
                nc.sync.dma_start(xs_g[:], xs_t[:, g, :])

                # X0^T h-major [256h x 1024n], relu + bias on ACT
                x0t = x0pool.tile([128, 2, N], f32r, tag="x0")
                for t in range(2):
                    for c in range(2):
                        ps = psx.tile([128, 512], f32, tag="psx", name="ps0")
                        nc.tensor.matmul(ps[:], wi_hi_sb[:, ts(t, 128)],
                                         xs_g[:, ts(c, 512)],
                                         start=True, stop=False)
                        nc.tensor.matmul(ps[:], wi_lo_sb[:, ts(t, 128)],
                                         xs_g[:, ts(c, 512)],
                                         start=False, stop=True)
                        nc.scalar.activation(x0t[:, t, ts(c, 512)], ps[:],
                                             Relu, bias=b_in_pp_sb[:, t:t + 1])

                x = x0t
                for layer in range(L):
                    # XW node-major: [1024m x 256h'], W as hi+lo f32r pair
                    xw = xwpool.tile([128, 8, H], f32r, tag="xw", name="xw")
                    for m in range(8):
                        ps = psw.tile([128, H], f32, tag="psw", name="psw")
                        k = 0
                        for t in range(2):
                            for w_sb in (w_hi_sb, w_lo_sb):
                                nc.tensor.matmul(
                                    ps[:], x[:, t, ts(m, 128)],
                                    w_sb[:, layer * 2 + t, :],
                                    start=(k == 0), stop=(k == 3))
                                k += 1
                        nc.vector.tensor_copy(xw[:, m, :], ps[:])

                    if layer < L - 1:
                        # X^T next = relu(XW^T A^T + b), h-major
                        xn = xpool.tile([128, 2, N], f32r, tag="xn", name="xn")
                        for t in range(2):
                            for c in range(2):
                                ps = psx.tile([128, 512], f32, tag="psx",
                                              name="psA")
                                for m in range(8):
                                    nc.tensor.matmul(
                                        ps[:], xw[:, m, ts(t, 128)],
                                        a_sb[:, m, ts(c, 512)],
                                        start=(m == 0), stop=(m == 7))
                                nc.scalar.activation(
                                    xn[:, t, ts(c, 512)], ps[:], Relu,
                                    bias=b_gcn_pp_sb[:, layer * 2 + t:
                                                     layer * 2 + t + 1])
                        x = xn
                    else:
                        # final layer: only the first K masked node tiles,
                        # node-major; tanh then residual via PE transpose
                        xf = xfpool.tile([128, Kmax, H], f32r, tag="xf",
                                         name="xf")
                        for c in range(K):
                            ps3 = psw.tile([128, H], f32, tag="psw",
                                           name="ps3")
                            for m in range(8):
                                nc.tensor.matmul(
                                    ps3[:], a_sb[:, m, ts(c, 128)],
                                    xw[:, m, :],
                                    start=(m == 0), stop=False)
                            nc.tensor.matmul(ps3[:], ones_sb[:],
                                             b_g3_row_sb[:],
                                             start=False, stop=True)
                            nc.scalar.activation(xf[:, c, :], ps3[:], Tanh)
                            for t in range(2):
                                pt = pstp.tile([128, 128], f32r, tag="pst",
                                               name="pt")
                                nc.tensor.transpose(
                                    pt[:], x0t[:, t, ts(c, 128)], ident_sb[:])
                                nc.vector.tensor_tensor(
                                    xf[:, c, ts(t, 128)],
                                    xf[:, c, ts(t, 128)], pt[:], add)

                # masked sums: psum[b, h] += M^T[n, b]^T @ Xf[n, h]
                pm = psm.tile([B, H], f32, tag="psm", name="pm")
                for c in range(K):
                    nc.tensor.matmul(pm[:], m_t_sb[:, off + c, :],
                                     xf[:, c, :], start=(c == 0),
                                     stop=(c == K - 1))
                if g == 0:
                    nc.vector.tensor_copy(out_acc[:], pm[:])
                else:
                    nc.vector.tensor_add(out_acc[:], out_acc[:], pm[:])

            # --- epilogue: divide by per-batch mask count (host-computed) ---
            out_sb = constp.tile([B, H], f32)
            nc.vector.tensor_scalar_mul(out_sb[:], out_acc[:], inv_sb[:])
            nc.sync.dma_start(out[:, :], out_sb[:])

    nc.compile()
    return nc


def _get_nc(NG, Ks):
    key = (NG, tuple(Ks))
    if key not in _CACHE:
        _CACHE[key] = _build_nc(NG, Ks)
    return _CACHE[key]


def _prepare_in_maps(cdfg_xs, cdfg_as, graph, coverpoint_mask,
                     W_in, b_in, W_gcn, b_gcn):
    cdfg_xs = np.asarray(cdfg_xs, dtype=np.float32)
    cdfg_as = np.asarray(cdfg_as, dtype=np.float32)
    graph = np.asarray(graph).astype(np.int64)
    maskf = np.asarray(coverpoint_mask).astype(np.float32)
    W_in = np.asarray(W_in, dtype=np.float32)
    b_in = np.asarray(b_in, dtype=np.float32)
    W_gcn = np.asarray(W_gcn, dtype=np.float32)
    b_gcn = np.asarray(b_gcn, dtype=np.float32)

    uniq = np.unique(graph)
    u = len(uniq)
    NG = max(1, (u + NCORES - 1) // NCORES)
    nslots = NG * NCORES

    # per-graph node permutation (union-masked nodes first) and tile count
    perms, kts = {}, {}
    for gid in uniq:
        um = maskf[graph == gid].any(axis=0)
        perms[int(gid)] = np.argsort(~um, kind="stable")
        kts[int(gid)] = max(1, int(np.ceil(um.sum() / 128)))

    # sort graphs by K desc; rank r -> (slot r//8, core r%8)
    order = sorted(uniq.tolist(), key=lambda g: -kts[int(g)])
    Ks = []
    for s in range(NG):
        bucket = [kts[int(order[r])] for r in range(s * 8, min((s + 1) * 8, u))]
        Ks.append(max(bucket) if bucket else 1)
    T = sum(Ks)
    offs = np.concatenate([[0], np.cumsum(Ks)]).astype(int)

    w_gcn_layout = np.ascontiguousarray(
        W_gcn.reshape(L, 2, 128, H).transpose(2, 0, 1, 3)
        .reshape(128, L * 2, H))
    w_gcn_hi = _rnd11(w_gcn_layout)
    w_gcn_lo = _rnd11(w_gcn_layout - w_gcn_hi)
    w_in_hi = _rnd11(W_in)
    w_in_lo = _rnd11(W_in - w_in_hi)

    common = {
        "w_in_hi": np.ascontiguousarray(w_in_hi),
        "w_in_lo": np.ascontiguousarray(w_in_lo),
        "w_gcn_hi": w_gcn_hi,
        "w_gcn_lo": w_gcn_lo,
        "b_in_pp": np.ascontiguousarray(b_in.reshape(2, 128).T),
        "b_gcn_pp": np.ascontiguousarray(
            b_gcn.reshape(L, 2, 128).transpose(2, 0, 1).reshape(128, L * 2)),
        "b_g3_row": np.ascontiguousarray(b_gcn[L - 1].reshape(1, H)),
        "ones_row": np.ones((1, 128), dtype=np.float32),
        "ident": np.eye(128, dtype=np.float32),
        "inv_cnt": np.ascontiguousarray(
            (1.0 / maskf.sum(axis=1, keepdims=True)).astype(np.float32)),
    }

    # per-graph prepped tensors (cached; dead slots reuse order[0])
    a_cache, xs_cache = {}, {}

    def graph_data(gid):
        if gid not in a_cache:
            p = perms[gid]
            a_cache[gid] = np.ascontiguousarray(
                cdfg_as[gid][p][:, p].T)
            xs_cache[gid] = np.ascontiguousarray(cdfg_xs[gid][p].T)
        return a_cache[gid], xs_cache[gid]

    in_maps = []
    for k in range(NCORES):
        a_t = np.empty((NG, N, N), dtype=np.float32)
        xs_t = np.empty((F, NG, N), dtype=np.float32)
        m_t = np.zeros((128, T, B), dtype=np.float32)
        for s in range(NG):
            r = s * 8 + k
            gid = int(order[r]) if r < u else int(order[0])
            a_g, xs_g = graph_data(gid)
            a_t[s] = a_g
            xs_t[:, s, :] = xs_g
            if r < u:
                p = perms[gid]
                rows = np.nonzero(graph == gid)[0]
                for b in rows:
                    mp = maskf[b][p]
                    for c in range(kts[gid]):
                        m_t[:, offs[s] + c, b] = mp[c * 128:(c + 1) * 128]
        in_maps.append({"a_t": a_t, "xs_t": xs_t,
                        "m_t": m_t, **common})
    meta = {"NG": NG, "Ks": Ks, "order": order, "u": u}
    return in_maps, meta


def _assemble_out(results, graph, meta):
    graph = np.asarray(graph).astype(np.int64)
    out = np.zeros((B, H), dtype=np.float32)
    for r in range(meta["u"]):
        s, k = r // 8, r % 8
        rows = graph == meta["order"][r]
        out[rows] = results[k]["out"][rows]
    return out


def kernel(cdfg_xs, cdfg_as, graph, coverpoint_mask, W_in, b_in, W_gcn, b_gcn):
    from concourse.bass_utils import run_bass_kernel_spmd

    in_maps, meta = _prepare_in_maps(
        cdfg_xs, cdfg_as, graph, coverpoint_mask, W_in, b_in, W_gcn, b_gcn)
    nc = _get_nc(meta["NG"], meta["Ks"])
    res = run_bass_kernel_spmd(nc, in_maps, core_ids=list(range(NCORES)))
    return _assemble_out(res.results, graph, meta)


# revision 8
# speedup vs baseline: 1.7569x; 1.0404x over previous
"""Trainium2 Bass kernel for CdfgReader GNN message passing.

Strategy:
  - 64 batch items draw from <=32 unique CDFGs: compute the GNN once per
    unique graph; distribute ceil(u/8) graph slots per core across 8 cores
    (SPMD, one compiled program specialized to the input's structure).
  - Error budget (tolerance 2e-2): the end-to-end error is dominated by the
    f32r rounding of the *weights* (a systematic perturbation); activation
    rounding averages out through the A-multiply and the masked mean.
    So W_in/W_gcn ship as f32r hi+lo pairs (every X@W does 2 matmuls per
    contraction tile), while activations stay single f32r and every A-mult
    runs once.  Measured end-to-end ~1.3e-3.
  - Per slot: X0^T = relu(W^T xs^T) h-major; 3x { XW node-major pipelined
    m-outer with the h-major A-multiply X^T = relu(XW^T A^T) }; the final
    layer is computed node-major only for the first K_g 128-node tiles,
    where the host permutes each graph's nodes so the union of its
    coverpoint masks comes first.  The residual relu(xs@W_in+b) is
    recomputed node-major for those K_g tiles directly from xs^T (cheap,
    and it fills the PE while the first slot's A matrix streams in), and
    the masked sums use a small mask matmul.  DMAs are ordered so the
    input-layer operands land first; A streams per 128-row chunk.
"""

import numpy as np

NCORES = 8
N = 1024        # max nodes
F = 128         # input feature dim
H = 256         # hidden dim
L = 4           # GCN layers
B = 64          # batch (coverpoints)

_CACHE = {}


def _rnd11(x):
    # round-to-nearest-even at 11 explicit mantissa bits (f32r-exact)
    m, e = np.frexp(np.float32(x))
    m = np.round(m * 4096.0) / 4096.0
    return np.ldexp(m, e).astype(np.float32)


def _build_nc(NG, Ks):
    import concourse.bass as bass  # noqa: F401
    import concourse.mybir as mybir
    import concourse.tile as tile
    from concourse import bacc
    from concourse.bass import ts

    f32 = mybir.dt.float32
    f32r = mybir.dt.float32r
    Relu = mybir.ActivationFunctionType.Relu
    Tanh = mybir.ActivationFunctionType.Tanh
    add = mybir.AluOpType.add

    T = sum(Ks)
    offs = np.concatenate([[0], np.cumsum(Ks)]).astype(int)
    Kmax = max(Ks)

    nc = bacc.Bacc("TRN2", target_bir_lowering=False, debug=False,
                   num_devices=NCORES)

    a_t = nc.dram_tensor("a_t", [NG, N, N], f32r, kind="ExternalInput")
    xs_t = nc.dram_tensor("xs_t", [F, NG, N], f32r, kind="ExternalInput")
    m_t = nc.dram_tensor("m_t", [128, T, B], f32r, kind="ExternalInput")
    w_in_hi = nc.dram_tensor("w_in_hi", [F, H], f32r, kind="ExternalInput")
    w_in_lo = nc.dram_tensor("w_in_lo", [F, H], f32r, kind="ExternalInput")
    w_gcn_hi = nc.dram_tensor("w_gcn_hi", [128, L * 2, H], f32r,
                              kind="ExternalInput")
    w_gcn_lo = nc.dram_tensor("w_gcn_lo", [128, L * 2, H], f32r,
                              kind="ExternalInput")
    b_in_pp = nc.dram_tensor("b_in_pp", [128, 2], f32, kind="ExternalInput")
    b_in_row = nc.dram_tensor("b_in_row", [1, H], f32r, kind="ExternalInput")
    b_gcn_pp = nc.dram_tensor("b_gcn_pp", [128, L * 2], f32,
                              kind="ExternalInput")
    b_g3_row = nc.dram_tensor("b_g3_row", [1, H], f32r, kind="ExternalInput")
    ones_row = nc.dram_tensor("ones_row", [1, 128], f32r, kind="ExternalInput")
    inv_cnt = nc.dram_tensor("inv_cnt", [B, 1], f32, kind="ExternalInput")
    out = nc.dram_tensor("out", [B, H], f32, kind="ExternalOutput")

    with tile.TileContext(nc) as tc:
        with (
            tc.tile_pool(name="const", bufs=1) as constp,
            tc.tile_pool(name="apool", bufs=2) as apool,
            tc.tile_pool(name="xspool", bufs=2) as xspool,
            tc.tile_pool(name="x0pool", bufs=2) as x0pool,
            tc.tile_pool(name="x0npool", bufs=2) as x0npool,
            tc.tile_pool(name="xpool", bufs=2) as xpool,
            tc.tile_pool(name="xwpool", bufs=2) as xwpool,
            tc.tile_pool(name="xfpool", bufs=2) as xfpool,
            tc.tile_pool(name="psx", bufs=4, space="PSUM") as psx,
            tc.tile_pool(name="psw", bufs=3, space="PSUM") as psw,
            tc.tile_pool(name="psm", bufs=1, space="PSUM") as psm,
        ):
            # --- DMA priority order: input-layer operands first, then the
            # first slot's A (chunked), then the rest.
            wi_hi_sb = constp.tile([128, H], f32r)
            nc.sync.dma_start(wi_hi_sb[:], w_in_hi[:, :])
            wi_lo_sb = constp.tile([128, H], f32r)
            nc.sync.dma_start(wi_lo_sb[:], w_in_lo[:, :])
            b_in_pp_sb = constp.tile([128, 2], f32)
            nc.sync.dma_start(b_in_pp_sb[:], b_in_pp[:, :])
            b_in_row_sb = constp.tile([1, H], f32r)
            nc.sync.dma_start(b_in_row_sb[:], b_in_row[:, :])
            ones_sb = constp.tile([1, 128], f32r)
            nc.sync.dma_start(ones_sb[:], ones_row[:, :])

            xs0 = xspool.tile([128, N], f32r, tag="xs", name="xs_g")
            nc.sync.dma_start(xs0[:], xs_t[:, 0, :])

            w_hi_sb = constp.tile([128, L * 2, H], f32r)
            w_lo_sb = constp.tile([128, L * 2, H], f32r)
            # layer-0 slices first (XW0 needs them before a_t finishes)
            nc.sync.dma_start(w_hi_sb[:, 0:2, :], w_gcn_hi[:, 0:2, :])
            nc.sync.dma_start(w_lo_sb[:, 0:2, :], w_gcn_lo[:, 0:2, :])

            a_sb0 = apool.tile([128, 8, N], f32r, tag="a", name="a_sb")
            for m in range(8):
                nc.sync.dma_start(a_sb0[:, m, :], a_t[0, ts(m, 128), :])

            nc.sync.dma_start(w_hi_sb[:, 2:8, :], w_gcn_hi[:, 2:8, :])
            nc.sync.dma_start(w_lo_sb[:, 2:8, :], w_gcn_lo[:, 2:8, :])
            b_gcn_pp_sb = constp.tile([128, L * 2], f32)
            nc.sync.dma_start(b_gcn_pp_sb[:], b_gcn_pp[:, :])
            b_g3_row_sb = constp.tile([1, H], f32r)
            nc.sync.dma_start(b_g3_row_sb[:], b_g3_row[:, :])
            m_t_sb = constp.tile([128, T, B], f32r)
            nc.sync.dma_start(m_t_sb[:], m_t[:, :, :])
            inv_sb = constp.tile([B, 1], f32)
            nc.sync.dma_start(inv_sb[:], inv_cnt[:, :])

            out_acc = constp.tile([B, H], f32)

            for g in range(NG):
                K = Ks[g]
                off = int(offs[g])
                if g == 0:
                    a_sb, xs_g = a_sb0, xs0
                else:
                    xs_g = xspool.tile([128, N], f32r, tag="xs", name="xs_g")
                    nc.sync.dma_start(xs_g[:], xs_t[:, g, :])
                    a_sb = apool.tile([128, 8, N], f32r, tag="a", name="a_sb")
                    for m in range(8):
                        nc.sync.dma_start(a_sb[:, m, :], a_t[g, ts(m, 128), :])

                # X0^T h-major [256h x 1024n], relu + bias on ACT
                x0t = x0pool.tile([128, 2, N], f32r, tag="x0")
                for t in range(2):
                    for c in range(2):
                        ps = psx.tile([128, 512], f32, tag="psx", name="ps0")
                        nc.tensor.matmul(ps[:], wi_hi_sb[:, ts(t, 128)],
                                         xs_g[:, ts(c, 512)],
                                         start=True, stop=False)
                        nc.tensor.matmul(ps[:], wi_lo_sb[:, ts(t, 128)],
                                         xs_g[:, ts(c, 512)],
                                         start=False, stop=True)
                        nc.scalar.activation(x0t[:, t, ts(c, 512)], ps[:],
                                             Relu, bias=b_in_pp_sb[:, t:t + 1])

                # residual X0 node-major for the K masked tiles, straight
                # from xs^T (fills the PE while slot 0's A streams in)
                x0n = x0npool.tile([128, Kmax, H], f32r, tag="x0n", name="x0n")
                for c in range(K):
                    ps = psw.tile([128, H], f32, tag="psw", name="ps0n")
                    nc.tensor.matmul(ps[:], xs_g[:, ts(c, 128)], wi_hi_sb[:],
                                     start=True, stop=False)
                    nc.tensor.matmul(ps[:], xs_g[:, ts(c, 128)], wi_lo_sb[:],
                                     start=False, stop=False)
                    nc.tensor.matmul(ps[:], ones_sb[:], b_in_row_sb[:],
                                     start=False, stop=True)
                    nc.scalar.activation(x0n[:, c, :], ps[:], Relu)

                x = x0t
                for layer in range(L - 1):
                    # XW node-major (W as hi+lo f32r pair), pipelined m-outer
                    # with the h-major A-multiply of the same layer
                    xw = xwpool.tile([128, 8, H], f32r, tag="xw", name="xw")
                    pss = [psx.tile([128, 512], f32, tag="psx",
                                    name=f"psA{tc_}")
                           for tc_ in range(4)]

                    def xw_group(m):
                        ps = psw.tile([128, H], f32, tag="psw", name="psw")
                        k = 0
                        for t in range(2):
                            for w_sb in (w_hi_sb, w_lo_sb):
                                nc.tensor.matmul(
                                    ps[:], x[:, t, ts(m, 128)],
                                    w_sb[:, layer * 2 + t, :],
                                    start=(k == 0), stop=(k == 3))
                                k += 1
                        nc.vector.tensor_copy(xw[:, m, :], ps[:])

                    def a_group(m):
                        for t in range(2):
                            for c in range(2):
                                nc.tensor.matmul(
                                    pss[t * 2 + c][:], xw[:, m, ts(t, 128)],
                                    a_sb[:, m, ts(c, 512)],
                                    start=(m == 0), stop=(m == 7))

                    xw_group(0)
                    for m in range(1, 8):
                        xw_group(m)
                        a_group(m - 1)
                    a_group(7)

                    xn = xpool.tile([128, 2, N], f32r, tag="xn", name="xn")
                    for t in range(2):
                        for c in range(2):
                            nc.scalar.activation(
                                xn[:, t, ts(c, 512)], pss[t * 2 + c][:], Relu,
                                bias=b_gcn_pp_sb[:, layer * 2 + t:
                                                 layer * 2 + t + 1])
                    x = xn

                # final layer: node-major, only the K masked tiles
                xw3 = xwpool.tile([128, 8, H], f32r, tag="xw", name="xw3")
                for m in range(8):
                    ps = psw.tile([128, H], f32, tag="psw", name="psw3")
                    k = 0
                    for t in range(2):
                        for w_sb in (w_hi_sb, w_lo_sb):
                            nc.tensor.matmul(
                                ps[:], x[:, t, ts(m, 128)],
                                w_sb[:, (L - 1) * 2 + t, :],
                                start=(k == 0), stop=(k == 3))
                            k += 1
                    nc.vector.tensor_copy(xw3[:, m, :], ps[:])

                xf = xfpool.tile([128, Kmax, H], f32r, tag="xf", name="xf")
                for c in range(K):
                    ps3 = psw.tile([128, H], f32, tag="psw", name="ps3")
                    for m in range(8):
                        nc.tensor.matmul(ps3[:], a_sb[:, m, ts(c, 128)],
                                         xw3[:, m, :],
                                         start=(m == 0), stop=False)
                    nc.tensor.matmul(ps3[:], ones_sb[:], b_g3_row_sb[:],
                                     start=False, stop=True)
                    nc.scalar.activation(xf[:, c, :], ps3[:], Tanh)
                    nc.vector.tensor_tensor(xf[:, c, :], xf[:, c, :],
                                            x0n[:, c, :], add)

                # masked sums: psum[b, h] += M^T[n, b]^T @ Xf[n, h]
                pm = psm.tile([B, H], f32, tag="psm", name="pm")
                for c in range(K):
                    nc.tensor.matmul(pm[:], m_t_sb[:, off + c, :],
                                     xf[:, c, :], start=(c == 0),
                                     stop=(c == K - 1))
                if g == 0:
                    nc.vector.tensor_copy(out_acc[:], pm[:])
                else:
                    nc.vector.tensor_add(out_acc[:], out_acc[:], pm[:])

            # --- epilogue: divide by per-batch mask count (host-computed) ---
            out_sb = constp.tile([B, H], f32)
            nc.vector.tensor_scalar_mul(out_sb[:], out_acc[:], inv_sb[:])
            nc.sync.dma_start(out[:, :], out_sb[:])

    nc.compile()
    return nc


def _get_nc(NG, Ks):
    key = (NG, tuple(Ks))
    if key not in _CACHE:
        _CACHE[key] = _build_nc(NG, Ks)
    return _CACHE[key]


def _prepare_in_maps(cdfg_xs, cdfg_as, graph, coverpoint_mask,
                     W_in, b_in, W_gcn, b_gcn):
    cdfg_xs = np.asarray(cdfg_xs, dtype=np.float32)
    cdfg_as = np.asarray(cdfg_as, dtype=np.float32)
    graph = np.asarray(graph).astype(np.int64)
    maskf = np.asarray(coverpoint_mask).astype(np.float32)
    W_in = np.asarray(W_in, dtype=np.float32)
    b_in = np.asarray(b_in, dtype=np.float32)
    W_gcn = np.asarray(W_gcn, dtype=np.float32)
    b_gcn = np.asarray(b_gcn, dtype=np.float32)

    uniq = np.unique(graph)
    u = len(uniq)
    NG = max(1, (u + NCORES - 1) // NCORES)

    # per-graph node permutation (union-masked nodes first) and tile count
    perms, kts = {}, {}
    for gid in uniq:
        um = maskf[graph == gid].any(axis=0)
        perms[int(gid)] = np.argsort(~um, kind="stable")
        kts[int(gid)] = max(1, int(np.ceil(um.sum() / 128)))

    # sort graphs by K desc; rank r -> (slot r//8, core r%8)
    order = sorted(uniq.tolist(), key=lambda g: -kts[int(g)])
    Ks = []
    for s in range(NG):
        bucket = [kts[int(order[r])] for r in range(s * 8, min((s + 1) * 8, u))]
        Ks.append(max(bucket) if bucket else 1)
    T = sum(Ks)
    offs = np.concatenate([[0], np.cumsum(Ks)]).astype(int)

    w_gcn_layout = np.ascontiguousarray(
        W_gcn.reshape(L, 2, 128, H).transpose(2, 0, 1, 3)
        .reshape(128, L * 2, H))
    w_gcn_hi = _rnd11(w_gcn_layout)
    w_gcn_lo = _rnd11(w_gcn_layout - w_gcn_hi)
    w_in_hi = _rnd11(W_in)
    w_in_lo = _rnd11(W_in - w_in_hi)

    common = {
        "w_in_hi": np.ascontiguousarray(w_in_hi),
        "w_in_lo": np.ascontiguousarray(w_in_lo),
        "w_gcn_hi": w_gcn_hi,
        "w_gcn_lo": w_gcn_lo,
        "b_in_pp": np.ascontiguousarray(b_in.reshape(2, 128).T),
        "b_in_row": np.ascontiguousarray(b_in.reshape(1, H)),
        "b_gcn_pp": np.ascontiguousarray(
            b_gcn.reshape(L, 2, 128).transpose(2, 0, 1).reshape(128, L * 2)),
        "b_g3_row": np.ascontiguousarray(b_gcn[L - 1].reshape(1, H)),
        "ones_row": np.ones((1, 128), dtype=np.float32),
        "inv_cnt": np.ascontiguousarray(
            (1.0 / maskf.sum(axis=1, keepdims=True)).astype(np.float32)),
    }

    # per-graph prepped tensors (cached; dead slots reuse order[0])
    a_cache, xs_cache = {}, {}

    def graph_data(gid):
        if gid not in a_cache:
            p = perms[gid]
            a_cache[gid] = np.ascontiguousarray(cdfg_as[gid][p][:, p].T)
            xs_cache[gid] = np.ascontiguousarray(cdfg_xs[gid][p].T)
        return a_cache[gid], xs_cache[gid]

    in_maps = []
    for k in range(NCORES):
        a_t = np.empty((NG, N, N), dtype=np.float32)
        xs_t = np.empty((F, NG, N), dtype=np.float32)
        m_t = np.zeros((128, T, B), dtype=np.float32)
        for s in range(NG):
            r = s * 8 + k
            gid = int(order[r]) if r < u else int(order[0])
            a_g, xs_g = graph_data(gid)
            a_t[s] = a_g
            xs_t[:, s, :] = xs_g
            if r < u:
                p = perms[gid]
                rows = np.nonzero(graph == gid)[0]
                for b in rows:
                    mp = maskf[b][p]
                    for c in range(kts[gid]):
                        m_t[:, offs[s] + c, b] = mp[c * 128:(c + 1) * 128]
        in_maps.append({"a_t": a_t, "xs_t": xs_t, "m_t": m_t, **common})
    meta = {"NG": NG, "Ks": Ks, "order": order, "u": u}
    return in_maps, meta


def _assemble_out(results, graph, meta):
    graph = np.asarray(graph).astype(np.int64)
    out = np.zeros((B, H), dtype=np.float32)
    for r in range(meta["u"]):
        s, k = r // 8, r % 8
        rows = graph == meta["order"][r]
        out[rows] = results[k]["out"][rows]
    return out


def kernel(cdfg_xs, cdfg_as, graph, coverpoint_mask, W_in, b_in, W_gcn, b_gcn):
    from concourse.bass_utils import run_bass_kernel_spmd

    in_maps, meta = _prepare_in_maps(
        cdfg_xs, cdfg_as, graph, coverpoint_mask, W_in, b_in, W_gcn, b_gcn)
    nc = _get_nc(meta["NG"], meta["Ks"])
    res = run_bass_kernel_spmd(nc, in_maps, core_ids=list(range(NCORES)))
    return _assemble_out(res.results, graph, meta)


# revision 10
# speedup vs baseline: 1.7844x; 1.0157x over previous
"""Trainium2 Bass kernel for CdfgReader GNN message passing.

Strategy:
  - 64 batch items draw from <=32 unique CDFGs: compute the GNN once per
    unique graph; distribute ceil(u/8) graph slots per core across 8 cores
    (SPMD, one compiled program specialized to the input's structure).
  - Error budget (tolerance 2e-2): the end-to-end error is dominated by the
    f32r rounding of the *weights* (a systematic perturbation); activation
    rounding averages out through the A-multiply and the masked mean.
    So W_in/W_gcn ship as f32r hi+lo pairs (every X@W does 2 matmuls per
    contraction tile), while activations stay single f32r and every A-mult
    runs once.  Measured end-to-end ~1.3e-3.
  - Per slot: X0^T = relu(W^T xs^T) h-major; 3x { XW node-major pipelined
    m-outer with the h-major A-multiply X^T = relu(XW^T A^T) }; the final
    layer is computed node-major only for the first K_g 128-node tiles,
    where the host permutes each graph's nodes so the union of its
    coverpoint masks comes first.  The residual relu(xs@W_in+b) is
    recomputed node-major for those K_g tiles directly from xs^T (cheap,
    and it fills the PE while the first slot's A matrix streams in), and
    the masked sums use a small mask matmul.  DMAs are ordered so the
    input-layer operands land first; A streams per 128-row chunk.
"""

import numpy as np

NCORES = 8
N = 1024        # max nodes
F = 128         # input feature dim
H = 256         # hidden dim
L = 4           # GCN layers
B = 64          # batch (coverpoints)

_CACHE = {}


def _rnd11(x):
    # round-to-nearest-even at 11 explicit mantissa bits (f32r-exact)
    m, e = np.frexp(np.float32(x))
    m = np.round(m * 4096.0) / 4096.0
    return np.ldexp(m, e).astype(np.float32)


def _build_nc(NG, Ks):
    import concourse.bass as bass  # noqa: F401
    import concourse.mybir as mybir
    import concourse.tile as tile
    from concourse import bacc
    from concourse.bass import ts

    f32 = mybir.dt.float32
    f32r = mybir.dt.float32r
    Relu = mybir.ActivationFunctionType.Relu
    Tanh = mybir.ActivationFunctionType.Tanh
    add = mybir.AluOpType.add

    T = sum(Ks)
    offs = np.concatenate([[0], np.cumsum(Ks)]).astype(int)
    Kmax = max(Ks)

    nc = bacc.Bacc("TRN2", target_bir_lowering=False, debug=False,
                   num_devices=NCORES)

    a_t = nc.dram_tensor("a_t", [NG, N, N], f32r, kind="ExternalInput")
    xs_t = nc.dram_tensor("xs_t", [F, NG, N], f32r, kind="ExternalInput")
    m_t = nc.dram_tensor("m_t", [128, T, B], f32r, kind="ExternalInput")
    w_in_hi = nc.dram_tensor("w_in_hi", [F, H], f32r, kind="ExternalInput")
    # W_gcn packed [128, (l,t), hi/lo, H] so layer slices are single DMAs
    w_gcn = nc.dram_tensor("w_gcn", [128, L * 2, 2, H], f32r,
                           kind="ExternalInput")
    b_pp = nc.dram_tensor("b_pp", [128, 2 + L * 2], f32, kind="ExternalInput")
    rows_c = nc.dram_tensor("rows_c", [1, 2 * H + 128], f32r,
                            kind="ExternalInput")
    inv_cnt = nc.dram_tensor("inv_cnt", [B, 1], f32, kind="ExternalInput")
    out = nc.dram_tensor("out", [B, H], f32, kind="ExternalOutput")

    with tile.TileContext(nc) as tc:
        with (
            tc.tile_pool(name="const", bufs=1) as constp,
            tc.tile_pool(name="apool", bufs=2) as apool,
            tc.tile_pool(name="xspool", bufs=2) as xspool,
            tc.tile_pool(name="x0pool", bufs=2) as x0pool,
            tc.tile_pool(name="x0npool", bufs=2) as x0npool,
            tc.tile_pool(name="xpool", bufs=2) as xpool,
            tc.tile_pool(name="xwpool", bufs=2) as xwpool,
            tc.tile_pool(name="xfpool", bufs=2) as xfpool,
            tc.tile_pool(name="psx", bufs=4, space="PSUM") as psx,
            tc.tile_pool(name="psw", bufs=3, space="PSUM") as psw,
            tc.tile_pool(name="psm", bufs=1, space="PSUM") as psm,
        ):
            # --- DMA priority order: input-layer operands first, then the
            # first slot's A (chunked), then the rest.
            wi_hi_sb = constp.tile([128, H], f32r)
            nc.sync.dma_start(wi_hi_sb[:], w_in_hi[:, :])
            b_pp_sb = constp.tile([128, 2 + L * 2], f32)
            nc.sync.dma_start(b_pp_sb[:], b_pp[:, :])
            rows_sb = constp.tile([1, 2 * H + 128], f32r)
            nc.sync.dma_start(rows_sb[:], rows_c[:, :])
            b_in_pp_sb = b_pp_sb[:, 0:2]
            b_gcn_pp_sb = b_pp_sb[:, 2:]
            b_in_row_sb = rows_sb[:, 0:H]
            b_g3_row_sb = rows_sb[:, H:2 * H]
            ones_sb = rows_sb[:, 2 * H:]

            xs0 = xspool.tile([128, N], f32r, tag="xs", name="xs_g")
            nc.sync.dma_start(xs0[:], xs_t[:, 0, :])

            w_sb = constp.tile([128, L * 2, 2, H], f32r)
            # layer-0 slices first (XW0 needs them before a_t finishes)
            nc.sync.dma_start(w_sb[:, 0:2, :, :], w_gcn[:, 0:2, :, :])

            a_sb0 = apool.tile([128, 8, N], f32r, tag="a", name="a_sb")
            for m in range(8):
                nc.sync.dma_start(a_sb0[:, m, :], a_t[0, ts(m, 128), :])

            nc.sync.dma_start(w_sb[:, 2:8, :, :], w_gcn[:, 2:8, :, :])
            m_t_sb = constp.tile([128, T, B], f32r)
            nc.sync.dma_start(m_t_sb[:], m_t[:, :, :])
            inv_sb = constp.tile([B, 1], f32)
            nc.sync.dma_start(inv_sb[:], inv_cnt[:, :])

            out_acc = constp.tile([B, H], f32)

            for g in range(NG):
                K = Ks[g]
                off = int(offs[g])
                if g == 0:
                    a_sb, xs_g = a_sb0, xs0
                else:
                    xs_g = xspool.tile([128, N], f32r, tag="xs", name="xs_g")
                    nc.sync.dma_start(xs_g[:], xs_t[:, g, :])
                    a_sb = apool.tile([128, 8, N], f32r, tag="a", name="a_sb")
                    for m in range(8):
                        nc.sync.dma_start(a_sb[:, m, :], a_t[g, ts(m, 128), :])

                # X0^T h-major [256h x 1024n], relu + bias on ACT
                x0t = x0pool.tile([128, 2, N], f32r, tag="x0")
                for t in range(2):
                    for c in range(2):
                        ps = psx.tile([128, 512], f32, tag="psx", name="ps0")
                        nc.tensor.matmul(ps[:], wi_hi_sb[:, ts(t, 128)],
                                         xs_g[:, ts(c, 512)],
                                         start=True, stop=True)
                        nc.scalar.activation(x0t[:, t, ts(c, 512)], ps[:],
                                             Relu, bias=b_in_pp_sb[:, t:t + 1])

                # residual X0 node-major for the K masked tiles, straight
                # from xs^T; emitted in pieces as PE filler (all upfront for
                # slot 0 -- it hides under the initial A DMA -- else spread
                # across layer boundaries to cover the ACT handoff)
                x0n = x0npool.tile([128, Kmax, H], f32r, tag="x0n", name="x0n")

                def x0n_group(c):
                    ps = psw.tile([128, H], f32, tag="psw", name="ps0n",
                                  bufs=2)
                    nc.tensor.matmul(ps[:], xs_g[:, ts(c, 128)], wi_hi_sb[:],
                                     start=True, stop=False)
                    nc.tensor.matmul(ps[:], ones_sb[:], b_in_row_sb[:],
                                     start=False, stop=True)
                    nc.scalar.activation(x0n[:, c, :], ps[:], Relu)

                if g == 0:
                    x0n_todo = [[], [], [], []]
                    for c in range(K):
                        x0n_group(c)
                else:
                    cs = list(range(K))
                    q = (K + 3) // 4
                    x0n_todo = [cs[0:q], cs[q:2 * q], cs[2 * q:3 * q],
                                cs[3 * q:]]
                    for c in x0n_todo[0]:
                        x0n_group(c)

                x = x0t
                for layer in range(L - 1):
                    # XW node-major (W as hi+lo f32r pair), pipelined m-outer
                    # with the h-major A-multiply of the same layer
                    xw = xwpool.tile([128, 8, H], f32r, tag="xw", name="xw")
                    pss = [psx.tile([128, 512], f32, tag="psx",
                                    name=f"psA{tc_}")
                           for tc_ in range(4)]

                    def xw_group(m):
                        ps = psw.tile([128, H], f32, tag="psw",
                                      name="psw", bufs=2)
                        k = 0
                        for t in range(2):
                            for hl in range(2):
                                nc.tensor.matmul(
                                    ps[:], x[:, t, ts(m, 128)],
                                    w_sb[:, layer * 2 + t, hl, :],
                                    start=(k == 0), stop=(k == 3))
                                k += 1
                        nc.vector.tensor_copy(xw[:, m, :], ps[:])

                    def a_group(m):
                        # m==7 closes the groups; finish the c=0 chunks
                        # first -- the next layer's XW m=0 waits on them
                        order = ([(0, 0), (1, 0), (0, 1), (1, 1)]
                                 if m == 7 else
                                 [(0, 0), (0, 1), (1, 0), (1, 1)])
                        for t, c in order:
                            nc.tensor.matmul(
                                pss[t * 2 + c][:], xw[:, m, ts(t, 128)],
                                a_sb[:, m, ts(c, 512)],
                                start=(m == 0), stop=(m == 7))

                    xw_group(0)
                    for m in range(1, 8):
                        xw_group(m)
                        a_group(m - 1)
                    a_group(7)

                    xn = xpool.tile([128, 2, N], f32r, tag="xn", name="xn")
                    for t, c in [(0, 0), (1, 0), (0, 1), (1, 1)]:
                        nc.scalar.activation(
                            xn[:, t, ts(c, 512)], pss[t * 2 + c][:], Relu,
                            bias=b_gcn_pp_sb[:, layer * 2 + t:
                                             layer * 2 + t + 1])
                    for c in x0n_todo[layer + 1]:
                        x0n_group(c)
                    x = xn

                # final layer: node-major, only the K masked tiles.
                # XW3 m-groups pipeline with the first c-group's A matmuls.
                xw3 = xwpool.tile([128, 8, H], f32r, tag="xw", name="xw3")
                xf = xfpool.tile([128, Kmax, H], f32r, tag="xf", name="xf")
                pm = psm.tile([B, H], f32, tag="psm", name="pm")

                def xw3_group(m):
                    ps = psw.tile([128, H], f32, tag="psw",
                                  name="psw3", bufs=2)
                    k = 0
                    for t in range(2):
                        for hl in range(2):
                            nc.tensor.matmul(
                                ps[:], x[:, t, ts(m, 128)],
                                w_sb[:, (L - 1) * 2 + t, hl, :],
                                start=(k == 0), stop=(k == 3))
                            k += 1
                    nc.vector.tensor_copy(xw3[:, m, :], ps[:])

                ps3s = {}

                def l3_mm(c, m):
                    if m == 0:
                        ps3s[c] = psw.tile([128, H], f32, tag="ps3",
                                           name="ps3", bufs=1)
                    nc.tensor.matmul(ps3s[c][:], a_sb[:, m, ts(c, 128)],
                                     xw3[:, m, :],
                                     start=(m == 0), stop=False)
                    if m == 7:
                        nc.tensor.matmul(ps3s[c][:], ones_sb[:],
                                         b_g3_row_sb[:],
                                         start=False, stop=True)
                        nc.scalar.activation(xf[:, c, :], ps3s[c][:], Tanh)
                        nc.vector.tensor_tensor(xf[:, c, :], xf[:, c, :],
                                                x0n[:, c, :], add)
                        nc.tensor.matmul(pm[:], m_t_sb[:, off + c, :],
                                         xf[:, c, :], start=(c == 0),
                                         stop=(c == K - 1))

                xw3_group(0)
                for m in range(1, 8):
                    xw3_group(m)
                    l3_mm(0, m - 1)
                l3_mm(0, 7)
                for c in range(1, K):
                    for m in range(8):
                        l3_mm(c, m)
                if g == 0:
                    nc.vector.tensor_copy(out_acc[:], pm[:])
                else:
                    nc.vector.tensor_add(out_acc[:], out_acc[:], pm[:])

            # --- epilogue: divide by per-batch mask count (host-computed) ---
            out_sb = constp.tile([B, H], f32)
            nc.vector.tensor_scalar_mul(out_sb[:], out_acc[:], inv_sb[:])
            nc.sync.dma_start(out[:, :], out_sb[:])

    nc.compile()
    return nc


def _get_nc(NG, Ks):
    key = (NG, tuple(Ks))
    if key not in _CACHE:
        _CACHE[key] = _build_nc(NG, Ks)
    return _CACHE[key]


def _prepare_in_maps(cdfg_xs, cdfg_as, graph, coverpoint_mask,
                     W_in, b_in, W_gcn, b_gcn):
    cdfg_xs = np.asarray(cdfg_xs, dtype=np.float32)
    cdfg_as = np.asarray(cdfg_as, dtype=np.float32)
    graph = np.asarray(graph).astype(np.int64)
    maskf = np.asarray(coverpoint_mask).astype(np.float32)
    W_in = np.asarray(W_in, dtype=np.float32)
    b_in = np.asarray(b_in, dtype=np.float32)
    W_gcn = np.asarray(W_gcn, dtype=np.float32)
    b_gcn = np.asarray(b_gcn, dtype=np.float32)

    uniq = np.unique(graph)
    u = len(uniq)
    NG = max(1, (u + NCORES - 1) // NCORES)

    # per-graph node permutation (union-masked nodes first) and tile count
    perms, kts = {}, {}
    for gid in uniq:
        um = maskf[graph == gid].any(axis=0)
        perms[int(gid)] = np.argsort(~um, kind="stable")
        kts[int(gid)] = max(1, int(np.ceil(um.sum() / 128)))

    # sort graphs by K desc; rank r -> (slot r//8, core r%8)
    order = sorted(uniq.tolist(), key=lambda g: -kts[int(g)])
    Ks = []
    for s in range(NG):
        bucket = [kts[int(order[r])] for r in range(s * 8, min((s + 1) * 8, u))]
        Ks.append(max(bucket) if bucket else 1)
    T = sum(Ks)
    offs = np.concatenate([[0], np.cumsum(Ks)]).astype(int)

    w_gcn_layout = np.ascontiguousarray(
        W_gcn.reshape(L, 2, 128, H).transpose(2, 0, 1, 3)
        .reshape(128, L * 2, H))
    w_gcn_hi = _rnd11(w_gcn_layout)
    w_gcn_lo = _rnd11(w_gcn_layout - w_gcn_hi)
    w_gcn_pack = np.ascontiguousarray(
        np.stack([w_gcn_hi, w_gcn_lo], axis=2))
    b_pp = np.concatenate([
        b_in.reshape(2, 128).T,
        b_gcn.reshape(L, 2, 128).transpose(2, 0, 1).reshape(128, L * 2)],
        axis=1)
    rows_c = np.concatenate([
        b_in.reshape(1, H), b_gcn[L - 1].reshape(1, H),
        np.ones((1, 128), dtype=np.float32)], axis=1)

    common = {
        "w_in_hi": np.ascontiguousarray(_rnd11(W_in)),
        "w_gcn": w_gcn_pack,
        "b_pp": np.ascontiguousarray(b_pp.astype(np.float32)),
        "rows_c": np.ascontiguousarray(rows_c.astype(np.float32)),
        "inv_cnt": np.ascontiguousarray(
            (1.0 / maskf.sum(axis=1, keepdims=True)).astype(np.float32)),
    }

    # per-graph prepped tensors (cached; dead slots reuse order[0])
    a_cache, xs_cache = {}, {}

    def graph_data(gid):
        if gid not in a_cache:
            p = perms[gid]
            a_cache[gid] = np.ascontiguousarray(cdfg_as[gid][p][:, p].T)
            xs_cache[gid] = np.ascontiguousarray(cdfg_xs[gid][p].T)
        return a_cache[gid], xs_cache[gid]

    in_maps = []
    for k in range(NCORES):
        a_t = np.empty((NG, N, N), dtype=np.float32)
        xs_t = np.empty((F, NG, N), dtype=np.float32)
        m_t = np.zeros((128, T, B), dtype=np.float32)
        for s in range(NG):
            r = s * 8 + k
            gid = int(order[r]) if r < u else int(order[0])
            a_g, xs_g = graph_data(gid)
            a_t[s] = a_g
            xs_t[:, s, :] = xs_g
            if r < u:
                p = perms[gid]
                rows = np.nonzero(graph == gid)[0]
                for b in rows:
                    mp = maskf[b][p]
                    for c in range(kts[gid]):
                        m_t[:, offs[s] + c, b] = mp[c * 128:(c + 1) * 128]
        in_maps.append({"a_t": a_t, "xs_t": xs_t, "m_t": m_t, **common})
    meta = {"NG": NG, "Ks": Ks, "order": order, "u": u}
    return in_maps, meta


def _assemble_out(results, graph, meta):
    graph = np.asarray(graph).astype(np.int64)
    out = np.zeros((B, H), dtype=np.float32)
    for r in range(meta["u"]):
        s, k = r // 8, r % 8
        rows = graph == meta["order"][r]
        out[rows] = results[k]["out"][rows]
    return out


def kernel(cdfg_xs, cdfg_as, graph, coverpoint_mask, W_in, b_in, W_gcn, b_gcn):
    from concourse.bass_utils import run_bass_kernel_spmd

    in_maps, meta = _prepare_in_maps(
        cdfg_xs, cdfg_as, graph, coverpoint_mask, W_in, b_in, W_gcn, b_gcn)
    nc = _get_nc(meta["NG"], meta["Ks"])
    res = run_bass_kernel_spmd(nc, in_maps, core_ids=list(range(NCORES)))
    return _assemble_out(res.results, graph, meta)


# revision 11
# speedup vs baseline: 1.8358x; 1.0288x over previous
"""Trainium2 Bass kernel for CdfgReader GNN message passing.

Strategy:
  - 64 batch items draw from <=32 unique CDFGs: compute the GNN once per
    unique graph; distribute ceil(u/8) graph slots per core across 8 cores
    (SPMD, one compiled program specialized to the input's structure).
  - Error budget (tolerance 2e-2): the end-to-end error is dominated by the
    f32r rounding of the *weights* (a systematic perturbation); activation
    rounding averages out through the A-multiply and the masked mean.
    So W_in/W_gcn ship as f32r hi+lo pairs (every X@W does 2 matmuls per
    contraction tile), while activations stay single f32r and every A-mult
    runs once.  Measured end-to-end ~1.3e-3.
  - Per slot: X0^T = relu(W^T xs^T) h-major; 3x { XW node-major pipelined
    m-outer with the h-major A-multiply X^T = relu(XW^T A^T) }; the final
    layer is computed node-major only for the first K_g 128-node tiles,
    where the host permutes each graph's nodes so the union of its
    coverpoint masks comes first.  The residual relu(xs@W_in+b) is
    recomputed node-major for those K_g tiles directly from xs^T (cheap,
    and it fills the PE while the first slot's A matrix streams in), and
    the masked sums use a small mask matmul.  DMAs are ordered so the
    input-layer operands land first; A streams per 128-row chunk.
"""

import numpy as np

NCORES = 8
N = 1024        # max nodes
F = 128         # input feature dim
H = 256         # hidden dim
L = 4           # GCN layers
B = 64          # batch (coverpoints)

_CACHE = {}


def _rnd11(x):
    # round-to-nearest-even at 11 explicit mantissa bits (f32r-exact)
    m, e = np.frexp(np.float32(x))
    m = np.round(m * 4096.0) / 4096.0
    return np.ldexp(m, e).astype(np.float32)


def _build_nc(NG, Ks):
    import concourse.bass as bass  # noqa: F401
    import concourse.mybir as mybir
    import concourse.tile as tile
    from concourse import bacc
    from concourse.bass import ts

    f32 = mybir.dt.float32
    f32r = mybir.dt.float32r
    Relu = mybir.ActivationFunctionType.Relu
    Tanh = mybir.ActivationFunctionType.Tanh
    add = mybir.AluOpType.add

    T = sum(Ks)
    offs = np.concatenate([[0], np.cumsum(Ks)]).astype(int)
    Kmax = max(Ks)

    nc = bacc.Bacc("TRN2", target_bir_lowering=False, debug=False,
                   num_devices=NCORES)

    a_t = nc.dram_tensor("a_t", [NG, N, N], f32r, kind="ExternalInput")
    xs_t = nc.dram_tensor("xs_t", [F, NG, N], f32r, kind="ExternalInput")
    m_t = nc.dram_tensor("m_t", [128, T, B], f32r, kind="ExternalInput")
    w_in_hi = nc.dram_tensor("w_in_hi", [F, H], f32r, kind="ExternalInput")
    # W_gcn packed [128, (l,t), hi/lo, H] so layer slices are single DMAs
    w_gcn = nc.dram_tensor("w_gcn", [128, L * 2, 2, H], f32r,
                           kind="ExternalInput")
    b_pp = nc.dram_tensor("b_pp", [128, 2 + L * 2], f32, kind="ExternalInput")
    rows_c = nc.dram_tensor("rows_c", [1, 2 * H + 128], f32r,
                            kind="ExternalInput")
    inv_cnt = nc.dram_tensor("inv_cnt", [B, 1], f32, kind="ExternalInput")
    out = nc.dram_tensor("out", [B, H], f32, kind="ExternalOutput")

    with tile.TileContext(nc) as tc:
        with (
            tc.tile_pool(name="const", bufs=1) as constp,
            tc.tile_pool(name="apool", bufs=2) as apool,
            tc.tile_pool(name="xspool", bufs=2) as xspool,
            tc.tile_pool(name="x0pool", bufs=2) as x0pool,
            tc.tile_pool(name="x0npool", bufs=2) as x0npool,
            tc.tile_pool(name="xpool", bufs=2) as xpool,
            tc.tile_pool(name="xwpool", bufs=2) as xwpool,
            tc.tile_pool(name="xfpool", bufs=2) as xfpool,
            tc.tile_pool(name="psx", bufs=3, space="PSUM") as psx,
            tc.tile_pool(name="psw", bufs=3, space="PSUM") as psw,
            tc.tile_pool(name="psm", bufs=1, space="PSUM") as psm,
        ):
            # --- DMA priority order: input-layer operands first, then the
            # first slot's A (chunked), then the rest.
            wi_hi_sb = constp.tile([128, H], f32r)
            nc.sync.dma_start(wi_hi_sb[:], w_in_hi[:, :])
            b_pp_sb = constp.tile([128, 2 + L * 2], f32)
            nc.sync.dma_start(b_pp_sb[:], b_pp[:, :])
            rows_sb = constp.tile([1, 2 * H + 128], f32r)
            nc.sync.dma_start(rows_sb[:], rows_c[:, :])
            b_in_pp_sb = b_pp_sb[:, 0:2]
            b_gcn_pp_sb = b_pp_sb[:, 2:]
            b_in_row_sb = rows_sb[:, 0:H]
            b_g3_row_sb = rows_sb[:, H:2 * H]
            ones_sb = rows_sb[:, 2 * H:]

            xs0 = xspool.tile([128, N], f32r, tag="xs", name="xs_g")
            nc.sync.dma_start(xs0[:], xs_t[:, 0, :])

            w_sb = constp.tile([128, L * 2, 2, H], f32r)
            # layer-0 slices first (XW0 needs them before a_t finishes)
            nc.sync.dma_start(w_sb[:, 0:2, :, :], w_gcn[:, 0:2, :, :])

            a_sb0 = apool.tile([128, 8, N], f32r, tag="a", name="a_sb")
            for m in range(8):
                nc.sync.dma_start(a_sb0[:, m, :], a_t[0, ts(m, 128), :])

            nc.sync.dma_start(w_sb[:, 2:8, :, :], w_gcn[:, 2:8, :, :])
            m_t_sb = constp.tile([128, T, B], f32r)
            nc.sync.dma_start(m_t_sb[:], m_t[:, :, :])
            inv_sb = constp.tile([B, 1], f32)
            nc.sync.dma_start(inv_sb[:], inv_cnt[:, :])

            out_acc = constp.tile([B, H], f32)

            for g in range(NG):
                K = Ks[g]
                off = int(offs[g])
                if g == 0:
                    a_sb, xs_g = a_sb0, xs0
                else:
                    xs_g = xspool.tile([128, N], f32r, tag="xs", name="xs_g")
                    nc.sync.dma_start(xs_g[:], xs_t[:, g, :])
                    a_sb = apool.tile([128, 8, N], f32r, tag="a", name="a_sb")
                    for m in range(8):
                        nc.sync.dma_start(a_sb[:, m, :], a_t[g, ts(m, 128), :])

                # X0^T h-major [256h x 1024n], relu + bias on ACT
                x0t = x0pool.tile([128, 2, N], f32r, tag="x0")
                for t in range(2):
                    for c in range(2):
                        ps = psx.tile([128, 512], f32, tag="psx", name="ps0")
                        nc.tensor.matmul(ps[:], wi_hi_sb[:, ts(t, 128)],
                                         xs_g[:, ts(c, 512)],
                                         start=True, stop=True)
                        nc.scalar.activation(x0t[:, t, ts(c, 512)], ps[:],
                                             Relu, bias=b_in_pp_sb[:, t:t + 1])

                # residual X0 node-major for the K masked tiles, straight
                # from xs^T; emitted in pieces as PE filler (all upfront for
                # slot 0 -- it hides under the initial A DMA -- else spread
                # across layer boundaries to cover the ACT handoff)
                x0n = x0npool.tile([128, Kmax, H], f32r, tag="x0n", name="x0n")

                def x0n_group(c):
                    ps = psw.tile([128, H], f32, tag="psw", name="ps0n",
                                  bufs=2)
                    nc.tensor.matmul(ps[:], xs_g[:, ts(c, 128)], wi_hi_sb[:],
                                     start=True, stop=False)
                    nc.tensor.matmul(ps[:], ones_sb[:], b_in_row_sb[:],
                                     start=False, stop=True)
                    nc.scalar.activation(x0n[:, c, :], ps[:], Relu)

                if g == 0:
                    x0n_todo = [[], [], [], []]
                    for c in range(K):
                        x0n_group(c)
                else:
                    cs = list(range(K))
                    q = (K + 3) // 4
                    x0n_todo = [cs[0:q], cs[q:2 * q], cs[2 * q:3 * q],
                                cs[3 * q:]]
                    for c in x0n_todo[0]:
                        x0n_group(c)

                x = x0t
                for layer in range(L - 1):
                    # XW node-major (W as hi+lo f32r pair).  The h-major
                    # A-multiply runs as two half-passes: pass A (c0 chunk)
                    # pipelines m-outer with the XW groups, pass B (c1)
                    # streams afterwards while the c0 ACTs drain, so the
                    # next layer's XW never waits on an ACT.
                    xw = xwpool.tile([128, 8, H], f32r, tag="xw", name="xw")
                    xn = xpool.tile([128, 2, N], f32r, tag="xn", name="xn")

                    def xw_group(m):
                        ps = psw.tile([128, H], f32, tag="psw",
                                      name="psw", bufs=2)
                        k = 0
                        for t in range(2):
                            for hl in range(2):
                                nc.tensor.matmul(
                                    ps[:], x[:, t, ts(m, 128)],
                                    w_sb[:, layer * 2 + t, hl, :],
                                    start=(k == 0), stop=(k == 3))
                                k += 1
                        nc.vector.tensor_copy(xw[:, m, :], ps[:])

                    pssA = [psx.tile([128, 512], f32, tag="psx",
                                     name=f"psA{t_}") for t_ in range(2)]

                    def a_pass(pss, c, m):
                        for t in range(2):
                            nc.tensor.matmul(
                                pss[t][:], xw[:, m, ts(t, 128)],
                                a_sb[:, m, ts(c, 512)],
                                start=(m == 0), stop=(m == 7))

                    xw_group(0)
                    for m in range(1, 8):
                        xw_group(m)
                        a_pass(pssA, 0, m - 1)
                    a_pass(pssA, 0, 7)
                    for t in range(2):
                        nc.scalar.activation(
                            xn[:, t, ts(0, 512)], pssA[t][:], Relu,
                            bias=b_gcn_pp_sb[:, layer * 2 + t:
                                             layer * 2 + t + 1])

                    pssB = [psx.tile([128, 512], f32, tag="psx",
                                     name=f"psB{t_}") for t_ in range(2)]
                    for m in range(8):
                        a_pass(pssB, 1, m)
                    for t in range(2):
                        nc.scalar.activation(
                            xn[:, t, ts(1, 512)], pssB[t][:], Relu,
                            bias=b_gcn_pp_sb[:, layer * 2 + t:
                                             layer * 2 + t + 1])
                    for c in x0n_todo[layer + 1]:
                        x0n_group(c)
                    x = xn

                # final layer: node-major, only the K masked tiles.
                # XW3 m-groups pipeline with the first c-group's A matmuls.
                xw3 = xwpool.tile([128, 8, H], f32r, tag="xw", name="xw3")
                xf = xfpool.tile([128, Kmax, H], f32r, tag="xf", name="xf")
                pm = psm.tile([B, H], f32, tag="psm", name="pm")

                def xw3_group(m):
                    ps = psw.tile([128, H], f32, tag="psw",
                                  name="psw3", bufs=2)
                    k = 0
                    for t in range(2):
                        for hl in range(2):
                            nc.tensor.matmul(
                                ps[:], x[:, t, ts(m, 128)],
                                w_sb[:, (L - 1) * 2 + t, hl, :],
                                start=(k == 0), stop=(k == 3))
                            k += 1
                    nc.vector.tensor_copy(xw3[:, m, :], ps[:])

                ps3s = {}

                def l3_mm(c, m):
                    if m == 0:
                        ps3s[c] = psw.tile([128, H], f32, tag="ps3",
                                           name="ps3", bufs=2)
                    nc.tensor.matmul(ps3s[c][:], a_sb[:, m, ts(c, 128)],
                                     xw3[:, m, :],
                                     start=(m == 0), stop=False)
                    if m == 7:
                        nc.tensor.matmul(ps3s[c][:], ones_sb[:],
                                         b_g3_row_sb[:],
                                         start=False, stop=True)
                        nc.scalar.activation(xf[:, c, :], ps3s[c][:], Tanh)
                        nc.vector.tensor_tensor(xf[:, c, :], xf[:, c, :],
                                                x0n[:, c, :], add)
                        nc.tensor.matmul(pm[:], m_t_sb[:, off + c, :],
                                         xf[:, c, :], start=(c == 0),
                                         stop=(c == K - 1))

                xw3_group(0)
                for m in range(1, 8):
                    xw3_group(m)
                    l3_mm(0, m - 1)
                l3_mm(0, 7)
                for c in range(1, K):
                    for m in range(8):
                        l3_mm(c, m)
                if g == 0:
                    nc.vector.tensor_copy(out_acc[:], pm[:])
                else:
                    nc.vector.tensor_add(out_acc[:], out_acc[:], pm[:])

            # --- epilogue: divide by per-batch mask count (host-computed) ---
            out_sb = constp.tile([B, H], f32)
            nc.vector.tensor_scalar_mul(out_sb[:], out_acc[:], inv_sb[:])
            nc.sync.dma_start(out[:, :], out_sb[:])

    nc.compile()
    return nc


def _get_nc(NG, Ks):
    key = (NG, tuple(Ks))
    if key not in _CACHE:
        _CACHE[key] = _build_nc(NG, Ks)
    return _CACHE[key]


def _prepare_in_maps(cdfg_xs, cdfg_as, graph, coverpoint_mask,
                     W_in, b_in, W_gcn, b_gcn):
    cdfg_xs = np.asarray(cdfg_xs, dtype=np.float32)
    cdfg_as = np.asarray(cdfg_as, dtype=np.float32)
    graph = np.asarray(graph).astype(np.int64)
    maskf = np.asarray(coverpoint_mask).astype(np.float32)
    W_in = np.asarray(W_in, dtype=np.float32)
    b_in = np.asarray(b_in, dtype=np.float32)
    W_gcn = np.asarray(W_gcn, dtype=np.float32)
    b_gcn = np.asarray(b_gcn, dtype=np.float32)

    uniq = np.unique(graph)
    u = len(uniq)
    NG = max(1, (u + NCORES - 1) // NCORES)

    # per-graph node permutation (union-masked nodes first) and tile count
    perms, kts = {}, {}
    for gid in uniq:
        um = maskf[graph == gid].any(axis=0)
        perms[int(gid)] = np.argsort(~um, kind="stable")
        kts[int(gid)] = max(1, int(np.ceil(um.sum() / 128)))

    # sort graphs by K desc; rank r -> (slot r//8, core r%8)
    order = sorted(uniq.tolist(), key=lambda g: -kts[int(g)])
    Ks = []
    for s in range(NG):
        bucket = [kts[int(order[r])] for r in range(s * 8, min((s + 1) * 8, u))]
        Ks.append(max(bucket) if bucket else 1)
    T = sum(Ks)
    offs = np.concatenate([[0], np.cumsum(Ks)]).astype(int)

    w_gcn_layout = np.ascontiguousarray(
        W_gcn.reshape(L, 2, 128, H).transpose(2, 0, 1, 3)
        .reshape(128, L * 2, H))
    w_gcn_hi = _rnd11(w_gcn_layout)
    w_gcn_lo = _rnd11(w_gcn_layout - w_gcn_hi)
    w_gcn_pack = np.ascontiguousarray(
        np.stack([w_gcn_hi, w_gcn_lo], axis=2))
    b_pp = np.concatenate([
        b_in.reshape(2, 128).T,
        b_gcn.reshape(L, 2, 128).transpose(2, 0, 1).reshape(128, L * 2)],
        axis=1)
    rows_c = np.concatenate([
        b_in.reshape(1, H), b_gcn[L - 1].reshape(1, H),
        np.ones((1, 128), dtype=np.float32)], axis=1)

    common = {
        "w_in_hi": np.ascontiguousarray(_rnd11(W_in)),
        "w_gcn": w_gcn_pack,
        "b_pp": np.ascontiguousarray(b_pp.astype(np.float32)),
        "rows_c": np.ascontiguousarray(rows_c.astype(np.float32)),
        "inv_cnt": np.ascontiguousarray(
            (1.0 / maskf.sum(axis=1, keepdims=True)).astype(np.float32)),
    }

    # per-graph prepped tensors (cached; dead slots reuse order[0])
    a_cache, xs_cache = {}, {}

    def graph_data(gid):
        if gid not in a_cache:
            p = perms[gid]
            a_cache[gid] = np.ascontiguousarray(cdfg_as[gid][p][:, p].T)
            xs_cache[gid] = np.ascontiguousarray(cdfg_xs[gid][p].T)
        return a_cache[gid], xs_cache[gid]

    in_maps = []
    for k in range(NCORES):
        a_t = np.empty((NG, N, N), dtype=np.float32)
        xs_t = np.empty((F, NG, N), dtype=np.float32)
        m_t = np.zeros((128, T, B), dtype=np.float32)
        for s in range(NG):
            r = s * 8 + k
            gid = int(order[r]) if r < u else int(order[0])
            a_g, xs_g = graph_data(gid)
            a_t[s] = a_g
            xs_t[:, s, :] = xs_g
            if r < u:
                p = perms[gid]
                rows = np.nonzero(graph == gid)[0]
                for b in rows:
                    mp = maskf[b][p]
                    for c in range(kts[gid]):
                        m_t[:, offs[s] + c, b] = mp[c * 128:(c + 1) * 128]
        in_maps.append({"a_t": a_t, "xs_t": xs_t, "m_t": m_t, **common})
    meta = {"NG": NG, "Ks": Ks, "order": order, "u": u}
    return in_maps, meta


def _assemble_out(results, graph, meta):
    graph = np.asarray(graph).astype(np.int64)
    out = np.zeros((B, H), dtype=np.float32)
    for r in range(meta["u"]):
        s, k = r // 8, r % 8
        rows = graph == meta["order"][r]
        out[rows] = results[k]["out"][rows]
    return out


def kernel(cdfg_xs, cdfg_as, graph, coverpoint_mask, W_in, b_in, W_gcn, b_gcn):
    from concourse.bass_utils import run_bass_kernel_spmd

    in_maps, meta = _prepare_in_maps(
        cdfg_xs, cdfg_as, graph, coverpoint_mask, W_in, b_in, W_gcn, b_gcn)
    nc = _get_nc(meta["NG"], meta["Ks"])
    res = run_bass_kernel_spmd(nc, in_maps, core_ids=list(range(NCORES)))
    return _assemble_out(res.results, graph, meta)
